# revision 8
# baseline (speedup 1.0000x reference)
"""Trainium2 Bass kernel for LocalDualDirectedMessagePassingLayer.

Strategy (8 cores, dest-sharded, fp8 DoubleRow):
  - Each core owns 1024 destination segments (8 blocks of 128 dests).
  - dest_seg is sorted, so each dest block's edges are contiguous; host pads
    each block's edge list to BCAP = ST_B*512 and packs fp8e4 operands:
      srcX [128,2,8,bcap] = concat(node_memory,node_features)[source_ids].T
                            as two 128-row K-tiles (DoubleRow layout)
      eftX [128,8,bcap]   = [edge_feat(64); time(32); ones; zeros(31)].T
  - Read MLP: ONE DoubleRow fp8 matmul per 512-edge supertile (K=256),
    relu on ACT writes fp8 srcT into the shared activation slab.
  - Msg MLP: per 128-edge tile ONE DoubleRow matmul with lhsT =
    [srcT;eft] slab slice, rhs = W_msg K-tiles; relu f32->bf16 split
    across DVE/Pool.
  - Aggregation: one-hot S built ON DEVICE per tile via DVE 4x-mode
    tensor_scalar(iota == ldest) * scale (scale = 1/cnt, 0 for padding);
    DoubleRow... S/msgs are bf16 so agg matmuls are plain bf16 pairs
    accumulating msg_mean^T [msg,dest] in PSUM across the block.
  - Per block: dst-side MLP chain (read/agg/upd/write) in bf16 ->
    tanh -> writeT [128,1024] f32, interleaved with next block.
  - Host: transpose writeT, scatter rows into a copy of node_memory.
"""

import sys

sys.path.insert(0, "/opt/trn_rl_repo")

import math

import ml_dtypes
import numpy as np

import concourse.bass as bass
import concourse.mybir as mybir
import concourse.tile as tile
from concourse import bacc
from concourse.bass_utils import run_bass_kernel_spmd

BF16 = ml_dtypes.bfloat16
FP8 = ml_dtypes.float8_e4m3
N_CORES = 8
SUP = 512
P = 128
N_DEST = 8192
D_MEM = 128

_PROG_CACHE: dict[int, object] = {}


def _build_program(st_b: int):
    """SPMD Bass program; BCAP = st_b*512 padded edges per dest block."""
    bcap = st_b * SUP
    e_cap = 8 * bcap
    nt = e_cap // P

    nc = bacc.Bacc("TRN2", target_bir_lowering=False, debug=False,
                   num_devices=N_CORES)
    f32 = mybir.dt.float32
    bf16 = mybir.dt.bfloat16
    fp8 = mybir.dt.float8e4
    AF = mybir.ActivationFunctionType
    OP = mybir.AluOpType
    DR = mybir.MatmulPerfMode.DoubleRow

    srcX = nc.dram_tensor("srcX", [P, 2, 8, bcap], fp8, kind="ExternalInput")
    eftX = nc.dram_tensor("eftX", [P, 8, bcap], fp8, kind="ExternalInput")
    S_d = nc.dram_tensor("S_d", [P, 8, 4 * st_b, P], fp8, kind="ExternalInput")
    invc = nc.dram_tensor("invc", [P, 1024], bf16, kind="ExternalInput")
    dstX = nc.dram_tensor("dstX", [P, 2, 1024], bf16, kind="ExternalInput")
    wrd = nc.dram_tensor("wrd", [P, 2, P], fp8, kind="ExternalInput")
    wmg = nc.dram_tensor("wmg", [P, 2, P], fp8, kind="ExternalInput")
    wrb = nc.dram_tensor("wrb", [P, 2, P], bf16, kind="ExternalInput")
    wa = nc.dram_tensor("wa", [P, 2, P], bf16, kind="ExternalInput")
    wu = nc.dram_tensor("wu", [P, 2, P], bf16, kind="ExternalInput")
    ww = nc.dram_tensor("ww", [P, P], bf16, kind="ExternalInput")
    br = nc.dram_tensor("br", [P, 1], f32, kind="ExternalInput")
    ba = nc.dram_tensor("ba", [P, 1], f32, kind="ExternalInput")
    bu = nc.dram_tensor("bu", [P, 1], f32, kind="ExternalInput")
    bw = nc.dram_tensor("bw", [P, 1], f32, kind="ExternalInput")
    out_d = nc.dram_tensor("writeT", [P, 1024], f32, kind="ExternalOutput")

    with tile.TileContext(nc) as tc:
        with (
            tc.tile_pool(name="const", bufs=1) as cp,
            tc.tile_pool(name="src", bufs=2) as iop,
            tc.tile_pool(name="act", bufs=2) as actp,
            tc.tile_pool(name="sp", bufs=4) as sp,
            tc.tile_pool(name="mp", bufs=4) as mp,
            tc.tile_pool(name="mid", bufs=8) as midp,
            tc.tile_pool(name="work", bufs=6, space="PSUM") as workps,
            tc.tile_pool(name="agg", bufs=1, space="PSUM") as aggps,
            tc.tile_pool(name="dst", bufs=1, space="PSUM") as dstps,
        ):
            def cload(ap, shape, dtype, tag, eng=nc.sync):
                t = cp.tile(shape, dtype, tag=tag)
                eng.dma_start(out=t[:], in_=ap)
                return t

            invc_t = cload(invc[:, :], [P, 1024], bf16, "invc", nc.scalar)
            dstX_t = cload(dstX[:, :, :], [P, 2, 1024], bf16, "dstX", nc.scalar)
            wrd_t = cload(wrd[:, :, :], [P, 2, P], fp8, "wrd")
            wmg_t = cload(wmg[:, :, :], [P, 2, P], fp8, "wmg")
            wrb_t = cload(wrb[:, :, :], [P, 2, P], bf16, "wrb")
            wa_t = cload(wa[:, :, :], [P, 2, P], bf16, "wa")
            wu_t = cload(wu[:, :, :], [P, 2, P], bf16, "wu")
            ww_t = cload(ww[:, :], [P, P], bf16, "ww")
            br_t = cload(br[:, :], [P, 1], f32, "br", nc.scalar)
            ba_t = cload(ba[:, :], [P, 1], f32, "ba", nc.scalar)
            bu_t = cload(bu[:, :], [P, 1], f32, "bu", nc.scalar)
            bw_t = cload(bw[:, :], [P, 1], f32, "bw", nc.scalar)

            def dst_stage(b, agg_ps, stage, hold):
                dc = slice(b * P, (b + 1) * P)
                if stage == 0:
                    drp = dstps.tile([P, P], f32, tag="dst")
                    nc.tensor.matmul(drp[:], lhsT=wrb_t[:, 0, :],
                                     rhs=dstX_t[:, 0, dc],
                                     start=True, stop=False)
                    nc.tensor.matmul(drp[:], lhsT=wrb_t[:, 1, :],
                                     rhs=dstX_t[:, 1, dc],
                                     start=False, stop=True)
                    dstr = midp.tile([P, P], bf16, tag="dstr")
                    nc.scalar.activation(dstr[:], drp[:], AF.Relu, bias=br_t[:, :1])
                    hold.update(dstr=dstr)
                elif stage == 1:
                    agp = dstps.tile([P, P], f32, tag="dst")
                    nc.tensor.matmul(agp[:], lhsT=wa_t[:, 0, :], rhs=hold["dstr"][:],
                                     start=True, stop=False)
                    nc.tensor.matmul(agp[:], lhsT=wa_t[:, 1, :], rhs=hold["mmean"][:],
                                     start=False, stop=True)
                    aggT = midp.tile([P, P], bf16, tag="aggT")
                    nc.scalar.activation(aggT[:], agp[:], AF.Relu, bias=ba_t[:, :1])
                    hold.update(aggT=aggT)
                elif stage == 2:
                    upp = dstps.tile([P, P], f32, tag="dst")
                    nc.tensor.matmul(upp[:], lhsT=wu_t[:, 0, :], rhs=hold["aggT"][:],
                                     start=True, stop=False)
                    nc.tensor.matmul(upp[:], lhsT=wu_t[:, 1, :], rhs=hold["dstr"][:],
                                     start=False, stop=True)
                    updT = midp.tile([P, P], bf16, tag="updT")
                    nc.scalar.activation(updT[:], upp[:], AF.Relu, bias=bu_t[:, :1])
                    hold.update(updT=updT)
                else:
                    wrp = dstps.tile([P, P], f32, tag="dst")
                    nc.tensor.matmul(wrp[:], lhsT=ww_t[:], rhs=hold["updT"][:],
                                     start=True, stop=True)
                    wout = midp.tile([P, P], f32, tag="wout")
                    nc.scalar.activation(wout[:], wrp[:], AF.Tanh, bias=bw_t[:, :1])
                    nc.sync.dma_start(out=out_d[:, dc], in_=wout[:])

            pending = None
            hold = {}
            for b in range(8):
                srcB = iop.tile([P, 2, bcap], fp8, tag="src")
                nc.sync.dma_start(out=srcB[:], in_=srcX[:, :, b, :])
                actB = actp.tile([P, 2, bcap], fp8, tag="act")
                nc.sync.dma_start(out=actB[:, 1, :], in_=eftX[:, b, :])
                S_blk = sp.tile([P, 4 * st_b, P], fp8, tag="S")
                nc.scalar.dma_start(out=S_blk[:], in_=S_d[:, b, :, :])
                agg_ps = aggps.tile([P, P], f32, tag="agg")
                for st in range(st_b):
                    cs = slice(st * SUP, (st + 1) * SUP)
                    t0 = (b * st_b + st) * 4

                    rd = workps.tile([P, SUP], f32, tag="work")
                    nc.tensor.matmul(rd[:], lhsT=wrd_t[:], rhs=srcB[:, :, cs],
                                     start=True, stop=True, perf_mode=DR)
                    nc.scalar.activation(actB[:, 0, cs], rd[:], AF.Relu,
                                         bias=br_t[:, :1])

                    mg = workps.tile([P, 4, P], f32, tag="work")
                    for q in range(4):
                        qs = slice((st * 4 + q) * P, (st * 4 + q + 1) * P)
                        nc.tensor.matmul(mg[:, q, :], lhsT=actB[:, :, qs],
                                         rhs=wmg_t[:],
                                         start=True, stop=True, perf_mode=DR)
                    msgs = mp.tile([P, 4, P], fp8, tag="msgs")
                    for h in range(2):
                        hs = slice(2 * h, 2 * h + 2)
                        nc.vector.tensor_scalar(
                            out=msgs[:, hs, :], in0=mg[:, hs, :],
                            scalar1=0.0, scalar2=None, op0=OP.max)
                        nc.tensor.matmul(agg_ps[:], lhsT=msgs[:, hs, :],
                                         rhs=S_blk[:, st * 4 + 2 * h:st * 4 + 2 * h + 2, :],
                                         start=(st == 0 and h == 0),
                                         stop=(st == st_b - 1 and h == 1),
                                         perf_mode=DR)

                    if pending is not None and st < 4:
                        dst_stage(pending[0], pending[1], st, hold)
                        if st == 3:
                            pending = None
                mmean = midp.tile([P, P], bf16, tag="mmean")
                nc.vector.tensor_tensor(mmean[:], agg_ps[:],
                                        invc_t[:, b * P:(b + 1) * P], OP.mult)
                pending = (b, agg_ps)
                hold = {"mmean": mmean}
            for stage in range(4):
                dst_stage(pending[0], pending[1], stage, hold)

    nc.finalize()
    return nc


def _prep_inputs(inputs):
    """Host-side shard/pack. Returns (in_maps, st_b, node_memory, node_ids)."""
    node_memory = np.ascontiguousarray(np.asarray(inputs["node_memory"], np.float32))
    node_features = np.asarray(inputs["node_features"], np.float32)
    edge_features = np.asarray(inputs["edge_features"], np.float32)
    time_encoding = np.asarray(inputs["time_encoding"], np.float32)
    node_ids = np.asarray(inputs["node_ids"]).astype(np.int64)
    source_ids = np.asarray(inputs["source_ids"]).astype(np.int64)
    edge_ids = np.asarray(inputs["edge_ids"]).astype(np.int64)
    dest_seg = np.asarray(inputs["dest_seg"]).astype(np.int64)
    W_read = np.asarray(inputs["W_read"], np.float32)
    b_read = np.asarray(inputs["b_read"], np.float32)
    W_msg = np.asarray(inputs["W_msg"], np.float32)
    b_msg = np.asarray(inputs["b_msg"], np.float32)
    W_agg = np.asarray(inputs["W_agg"], np.float32)
    b_agg = np.asarray(inputs["b_agg"], np.float32)
    W_upd = np.asarray(inputs["W_upd"], np.float32)
    b_upd = np.asarray(inputs["b_upd"], np.float32)
    W_write = np.asarray(inputs["W_write"], np.float32)
    b_write = np.asarray(inputs["b_write"], np.float32)

    n_edge = dest_seg.shape[0]

    cnt = np.bincount(dest_seg, minlength=N_DEST)
    inv_cnt = np.zeros(N_DEST, np.float32)
    nz = cnt > 0
    inv_cnt[nz] = 1.0 / cnt[nz]

    # 64 global dest blocks of 128; block B's edges are dest_seg in [B*128,(B+1)*128)
    bounds = np.searchsorted(dest_seg, np.arange(0, N_DEST + 1, P))
    per_block = np.diff(bounds)
    st_b = max(4, math.ceil(per_block.max() / SUP))
    bcap = st_b * SUP
    e_cap = 8 * bcap
    nt = e_cap // P

    # per-core edge selection (padded); esel indexes into the edge arrays
    esel = np.zeros((N_CORES, e_cap), np.int64)
    valid = np.zeros((N_CORES, e_cap), bool)
    for c in range(N_CORES):
        for blk in range(8):
            B = c * 8 + blk
            lo, hi = int(bounds[B]), int(bounds[B + 1])
            off = blk * bcap
            esel[c, off:off + hi - lo] = np.arange(lo, hi)
            valid[c, off:off + hi - lo] = True
    esel_f = esel.reshape(-1)
    valid_f = valid.reshape(-1)

    nodecat = np.concatenate([node_memory, node_features], axis=1)  # [N,256]

    src_rows = nodecat[source_ids[esel_f]]
    src_rows[~valid_f] = 0.0
    # srcX[c, p, k, b, j] = src_rows[c, b*bcap+j, k*128+p]
    srcX = np.ascontiguousarray(
        src_rows.reshape(N_CORES, 8, bcap, 2, P).transpose(0, 4, 3, 1, 2)
    ).astype(FP8)

    ef_rows = edge_features[edge_ids[esel_f]]
    t_rows = time_encoding[np.minimum(esel_f, n_edge - 1)]
    eft = np.zeros((len(esel_f), P), np.float32)
    eft[:, :64] = ef_rows
    eft[:, 64:96] = t_rows
    eft[valid_f, 96] = 1.0
    eft[~valid_f, :96] = 0.0
    eftX = np.ascontiguousarray(
        eft.reshape(N_CORES, 8, bcap, P).transpose(0, 3, 1, 2)).astype(FP8)

    ld_e = (dest_seg[esel_f] % P).astype(np.int64)
    ld_e[~valid_f] = 255
    S_flat = (ld_e[:, None] == np.arange(P)[None, :]).astype(FP8)
    S_pack = np.ascontiguousarray(
        S_flat.reshape(N_CORES, 8, 4 * st_b, P, P).transpose(0, 3, 1, 2, 4))

    invc = np.ascontiguousarray(np.broadcast_to(
        inv_cnt.reshape(N_CORES, 1, 1024), (N_CORES, P, 1024))).astype(BF16)

    drows = nodecat[node_ids]                                  # [8192, 256]
    dstX = np.ascontiguousarray(
        drows.reshape(N_CORES, 1024, 2, P).transpose(0, 3, 2, 1)).astype(BF16)

    wrd_h = np.ascontiguousarray(
        W_read.reshape(2, P, P).transpose(1, 0, 2)).astype(FP8)
    wmg_k1 = np.zeros((P, P), np.float32)
    wmg_k1[:64] = W_msg[128:192]
    wmg_k1[64:96] = W_msg[192:224]
    wmg_k1[96] = b_msg
    wmg_h = np.ascontiguousarray(
        np.stack([W_msg[:P], wmg_k1], axis=0).transpose(1, 0, 2)).astype(FP8)
    wrb_h = np.ascontiguousarray(
        W_read.reshape(2, P, P).transpose(1, 0, 2)).astype(BF16)
    wa_h = np.ascontiguousarray(
        W_agg.reshape(2, P, P).transpose(1, 0, 2)).astype(BF16)
    wu_h = np.ascontiguousarray(
        W_upd.reshape(2, P, P).transpose(1, 0, 2)).astype(BF16)
    ww_h = np.ascontiguousarray(W_write).astype(BF16)
    br_h = np.ascontiguousarray(b_read[:, None]).astype(np.float32)
    ba_h = np.ascontiguousarray(b_agg[:, None]).astype(np.float32)
    bu_h = np.ascontiguousarray(b_upd[:, None]).astype(np.float32)
    bw_h = np.ascontiguousarray(b_write[:, None]).astype(np.float32)

    in_maps = []
    for c in range(N_CORES):
        in_maps.append({
            "srcX": srcX[c], "eftX": eftX[c], "S_d": S_pack[c],
            "invc": invc[c], "dstX": dstX[c],
            "wrd": wrd_h, "wmg": wmg_h, "wrb": wrb_h, "wa": wa_h, "wu": wu_h,
            "ww": ww_h, "br": br_h, "ba": ba_h, "bu": bu_h, "bw": bw_h,
        })
    return in_maps, st_b, node_memory, node_ids


def run(inputs, trace=False, **kw):
    in_maps, st_b, node_memory, node_ids = _prep_inputs(inputs)
    if st_b not in _PROG_CACHE:
        _PROG_CACHE[st_b] = _build_program(st_b)
    nc = _PROG_CACHE[st_b]
    res = run_bass_kernel_spmd(nc, in_maps, core_ids=list(range(N_CORES)),
                               trace=trace, **kw)
    wt = np.concatenate(
        [np.asarray(res.results[c]["writeT"], np.float32).T
         for c in range(N_CORES)], axis=0)             # [8192, 128]
    out = node_memory.copy()
    out[node_ids] = wt
    return out, res


def kernel(**inputs) -> np.ndarray:
    out, _ = run(inputs, trace=False)
    return out


# revision 10
# speedup vs baseline: 1.5130x; 1.5130x over previous
"""Trainium2 Bass kernel for LocalDualDirectedMessagePassingLayer.

Strategy (8 cores, dest-sharded, fp8 DoubleRow):
  - Each core owns 1024 destination segments (8 blocks of 128 dests).
  - dest_seg is sorted, so each dest block's edges are contiguous; host pads
    each block's edge list to BCAP = ST_B*512 and packs fp8e4 operands:
      srcX [128,2,8,bcap] = concat(node_memory,node_features)[source_ids].T
                            as two 128-row K-tiles (DoubleRow layout)
      eftX [128,8,bcap]   = [edge_feat(64); time(32); ones; zeros(31)].T
  - Read MLP: ONE DoubleRow fp8 matmul per 512-edge supertile (K=256),
    relu on ACT writes fp8 srcT into the shared activation slab.
  - Msg MLP: per 128-edge tile ONE DoubleRow matmul with lhsT =
    [srcT;eft] slab slice, rhs = W_msg K-tiles; relu f32->bf16 split
    across DVE/Pool.
  - Aggregation: one-hot S built ON DEVICE per tile via DVE 4x-mode
    tensor_scalar(iota == ldest) * scale (scale = 1/cnt, 0 for padding);
    DoubleRow... S/msgs are bf16 so agg matmuls are plain bf16 pairs
    accumulating msg_mean^T [msg,dest] in PSUM across the block.
  - Per block: dst-side MLP chain (read/agg/upd/write) in bf16 ->
    tanh -> writeT [128,1024] f32, interleaved with next block.
  - Host: transpose writeT, scatter rows into a copy of node_memory.
"""

import sys

sys.path.insert(0, "/opt/trn_rl_repo")

import math

import ml_dtypes
import numpy as np

import concourse.bass as bass
import concourse.mybir as mybir
import concourse.tile as tile
from concourse import bacc
from concourse.bass_utils import run_bass_kernel_spmd

BF16 = ml_dtypes.bfloat16
FP8 = ml_dtypes.float8_e4m3
N_CORES = 8
SUP = 512
P = 128
N_DEST = 8192
D_MEM = 128

_PROG_CACHE: dict[int, object] = {}


def _build_program(st_b: int):
    """SPMD Bass program; BCAP = st_b*512 padded edges per dest block."""
    bcap = st_b * SUP
    e_cap = 8 * bcap
    nt = e_cap // P

    nc = bacc.Bacc("TRN2", target_bir_lowering=False, debug=False,
                   num_devices=N_CORES)
    f32 = mybir.dt.float32
    bf16 = mybir.dt.bfloat16
    fp8 = mybir.dt.float8e4
    AF = mybir.ActivationFunctionType
    OP = mybir.AluOpType
    DR = mybir.MatmulPerfMode.DoubleRow

    srcX = nc.dram_tensor("srcX", [P, 2, 8, bcap], fp8, kind="ExternalInput")
    eftX = nc.dram_tensor("eftX", [P, 8, bcap], fp8, kind="ExternalInput")
    S_d = nc.dram_tensor("S_d", [P, 8, 4 * st_b, P], fp8, kind="ExternalInput")
    invc = nc.dram_tensor("invc", [P, 1024], bf16, kind="ExternalInput")
    dstX = nc.dram_tensor("dstX", [P, 2, 1024], bf16, kind="ExternalInput")
    wrd = nc.dram_tensor("wrd", [P, 2, P], fp8, kind="ExternalInput")
    wmg = nc.dram_tensor("wmg", [P, 2, P], fp8, kind="ExternalInput")
    wrb = nc.dram_tensor("wrb", [P, 2, P], bf16, kind="ExternalInput")
    wa = nc.dram_tensor("wa", [P, 2, P], bf16, kind="ExternalInput")
    wu = nc.dram_tensor("wu", [P, 2, P], bf16, kind="ExternalInput")
    ww = nc.dram_tensor("ww", [P, P], bf16, kind="ExternalInput")
    br = nc.dram_tensor("br", [P, 1], f32, kind="ExternalInput")
    ba = nc.dram_tensor("ba", [P, 1], f32, kind="ExternalInput")
    bu = nc.dram_tensor("bu", [P, 1], f32, kind="ExternalInput")
    bw = nc.dram_tensor("bw", [P, 1], f32, kind="ExternalInput")
    out_d = nc.dram_tensor("writeT", [P, 1024], f32, kind="ExternalOutput")

    with tile.TileContext(nc) as tc:
        with (
            tc.tile_pool(name="const", bufs=1) as cp,
            tc.tile_pool(name="src", bufs=3) as iop,
            tc.tile_pool(name="act", bufs=3) as actp,
            tc.tile_pool(name="sp", bufs=3) as sp,
            tc.tile_pool(name="mp", bufs=4) as mp,
            tc.tile_pool(name="mid", bufs=8) as midp,
            tc.tile_pool(name="work", bufs=6, space="PSUM") as workps,
            tc.tile_pool(name="agg", bufs=1, space="PSUM") as aggps,
            tc.tile_pool(name="dst", bufs=1, space="PSUM") as dstps,
        ):
            def cload(ap, shape, dtype, tag, eng=nc.sync):
                t = cp.tile(shape, dtype, tag=tag)
                eng.dma_start(out=t[:], in_=ap)
                return t

            invc_t = cload(invc[:, :], [P, 1024], bf16, "invc", nc.scalar)
            dstX_t = cload(dstX[:, :, :], [P, 2, 1024], bf16, "dstX", nc.scalar)
            wrd_t = cload(wrd[:, :, :], [P, 2, P], fp8, "wrd")
            wmg_t = cload(wmg[:, :, :], [P, 2, P], fp8, "wmg")
            wrb_t = cload(wrb[:, :, :], [P, 2, P], bf16, "wrb")
            wa_t = cload(wa[:, :, :], [P, 2, P], bf16, "wa")
            wu_t = cload(wu[:, :, :], [P, 2, P], bf16, "wu")
            ww_t = cload(ww[:, :], [P, P], bf16, "ww")
            br_t = cload(br[:, :], [P, 1], f32, "br", nc.scalar)
            ba_t = cload(ba[:, :], [P, 1], f32, "ba", nc.scalar)
            bu_t = cload(bu[:, :], [P, 1], f32, "bu", nc.scalar)
            bw_t = cload(bw[:, :], [P, 1], f32, "bw", nc.scalar)

            def dst_stage(b, agg_ps, stage, hold):
                dc = slice(b * P, (b + 1) * P)
                if stage == 0:
                    drp = dstps.tile([P, P], f32, tag="dst")
                    nc.tensor.matmul(drp[:], lhsT=wrb_t[:, 0, :],
                                     rhs=dstX_t[:, 0, dc],
                                     start=True, stop=False)
                    nc.tensor.matmul(drp[:], lhsT=wrb_t[:, 1, :],
                                     rhs=dstX_t[:, 1, dc],
                                     start=False, stop=True)
                    dstr = midp.tile([P, P], bf16, tag="dstr")
                    nc.scalar.activation(dstr[:], drp[:], AF.Relu, bias=br_t[:, :1])
                    hold.update(dstr=dstr)
                elif stage == 1:
                    agp = dstps.tile([P, P], f32, tag="dst")
                    nc.tensor.matmul(agp[:], lhsT=wa_t[:, 0, :], rhs=hold["dstr"][:],
                                     start=True, stop=False)
                    nc.tensor.matmul(agp[:], lhsT=wa_t[:, 1, :], rhs=hold["mmean"][:],
                                     start=False, stop=True)
                    aggT = midp.tile([P, P], bf16, tag="aggT")
                    nc.scalar.activation(aggT[:], agp[:], AF.Relu, bias=ba_t[:, :1])
                    hold.update(aggT=aggT)
                elif stage == 2:
                    upp = dstps.tile([P, P], f32, tag="dst")
                    nc.tensor.matmul(upp[:], lhsT=wu_t[:, 0, :], rhs=hold["aggT"][:],
                                     start=True, stop=False)
                    nc.tensor.matmul(upp[:], lhsT=wu_t[:, 1, :], rhs=hold["dstr"][:],
                                     start=False, stop=True)
                    updT = midp.tile([P, P], bf16, tag="updT")
                    nc.scalar.activation(updT[:], upp[:], AF.Relu, bias=bu_t[:, :1])
                    hold.update(updT=updT)
                else:
                    wrp = dstps.tile([P, P], f32, tag="dst")
                    nc.tensor.matmul(wrp[:], lhsT=ww_t[:], rhs=hold["updT"][:],
                                     start=True, stop=True)
                    wout = midp.tile([P, P], f32, tag="wout")
                    nc.scalar.activation(wout[:], wrp[:], AF.Tanh, bias=bw_t[:, :1])
                    nc.sync.dma_start(out=out_d[:, dc], in_=wout[:])

            nsup = 8 * st_b
            blocks = {}

            def load_block(b):
                srcB = iop.tile([P, 2, bcap], fp8, tag="src")
                nc.sync.dma_start(out=srcB[:], in_=srcX[:, :, b, :])
                actB = actp.tile([P, 2, bcap], fp8, tag="act")
                nc.sync.dma_start(out=actB[:, 1, :], in_=eftX[:, b, :])
                S_blk = sp.tile([P, 4 * st_b, P], fp8, tag="S")
                nc.scalar.dma_start(out=S_blk[:], in_=S_d[:, b, :, :])
                blocks[b] = (srcB, actB, S_blk)

            def read_stage(g):
                """read MLP matmul + relu->fp8 for global supertile g."""
                b, st = divmod(g, st_b)
                cs = slice(st * SUP, (st + 1) * SUP)
                srcB, actB, _ = blocks[b]
                rd = workps.tile([P, SUP], f32, tag="work")
                nc.tensor.matmul(rd[:], lhsT=wrd_t[:], rhs=srcB[:, :, cs],
                                 start=True, stop=True, perf_mode=DR)
                nc.scalar.activation(actB[:, 0, cs], rd[:], AF.Relu,
                                     bias=br_t[:, :1])

            load_block(0)
            load_block(1)
            read_stage(0)
            pending = None
            hold = {}
            agg_ps = None
            for g in range(nsup):
                b, st = divmod(g, st_b)
                if st == 0:
                    agg_ps = aggps.tile([P, P], f32, tag="agg")
                    if b + 2 < 8:
                        load_block(b + 2)
                # software pipeline: next supertile's read+relu first, so
                # srT is ready a full iteration before the msg matmuls
                if g + 1 < nsup:
                    read_stage(g + 1)

                _, actB, S_blk = blocks[b]
                mg = workps.tile([P, 4, P], f32, tag="work")
                for q in range(4):
                    qs = slice((st * 4 + q) * P, (st * 4 + q + 1) * P)
                    nc.tensor.matmul(mg[:, q, :], lhsT=actB[:, :, qs],
                                     rhs=wmg_t[:],
                                     start=True, stop=True, perf_mode=DR)
                msgs = mp.tile([P, 4, P], fp8, tag="msgs")
                for h in range(2):
                    hs = slice(2 * h, 2 * h + 2)
                    nc.vector.tensor_scalar(
                        out=msgs[:, hs, :], in0=mg[:, hs, :],
                        scalar1=0.0, scalar2=None, op0=OP.max)
                    nc.tensor.matmul(agg_ps[:], lhsT=msgs[:, hs, :],
                                     rhs=S_blk[:, st * 4 + 2 * h:st * 4 + 2 * h + 2, :],
                                     start=(st == 0 and h == 0),
                                     stop=(st == st_b - 1 and h == 1),
                                     perf_mode=DR)

                if pending is not None and st < 4:
                    dst_stage(pending[0], pending[1], st, hold)
                    if st == 3:
                        pending = None
                if st == st_b - 1:
                    mmean = midp.tile([P, P], bf16, tag="mmean")
                    nc.vector.tensor_tensor(mmean[:], agg_ps[:],
                                            invc_t[:, b * P:(b + 1) * P],
                                            OP.mult)
                    pending = (b, agg_ps)
                    hold = {"mmean": mmean}
            for stage in range(4):
                dst_stage(pending[0], pending[1], stage, hold)

    nc.finalize()
    return nc


def _prep_inputs(inputs):
    """Host-side shard/pack. Returns (in_maps, st_b, node_memory, node_ids)."""
    node_memory = np.ascontiguousarray(np.asarray(inputs["node_memory"], np.float32))
    node_features = np.asarray(inputs["node_features"], np.float32)
    edge_features = np.asarray(inputs["edge_features"], np.float32)
    time_encoding = np.asarray(inputs["time_encoding"], np.float32)
    node_ids = np.asarray(inputs["node_ids"]).astype(np.int64)
    source_ids = np.asarray(inputs["source_ids"]).astype(np.int64)
    edge_ids = np.asarray(inputs["edge_ids"]).astype(np.int64)
    dest_seg = np.asarray(inputs["dest_seg"]).astype(np.int64)
    W_read = np.asarray(inputs["W_read"], np.float32)
    b_read = np.asarray(inputs["b_read"], np.float32)
    W_msg = np.asarray(inputs["W_msg"], np.float32)
    b_msg = np.asarray(inputs["b_msg"], np.float32)
    W_agg = np.asarray(inputs["W_agg"], np.float32)
    b_agg = np.asarray(inputs["b_agg"], np.float32)
    W_upd = np.asarray(inputs["W_upd"], np.float32)
    b_upd = np.asarray(inputs["b_upd"], np.float32)
    W_write = np.asarray(inputs["W_write"], np.float32)
    b_write = np.asarray(inputs["b_write"], np.float32)

    n_edge = dest_seg.shape[0]

    cnt = np.bincount(dest_seg, minlength=N_DEST)
    inv_cnt = np.zeros(N_DEST, np.float32)
    nz = cnt > 0
    inv_cnt[nz] = 1.0 / cnt[nz]

    # 64 global dest blocks of 128; block B's edges are dest_seg in [B*128,(B+1)*128)
    bounds = np.searchsorted(dest_seg, np.arange(0, N_DEST + 1, P))
    per_block = np.diff(bounds)
    st_b = max(4, math.ceil(per_block.max() / SUP))
    bcap = st_b * SUP
    e_cap = 8 * bcap
    nt = e_cap // P

    # per-core edge selection (padded); esel indexes into the edge arrays
    esel = np.zeros((N_CORES, e_cap), np.int64)
    valid = np.zeros((N_CORES, e_cap), bool)
    for c in range(N_CORES):
        for blk in range(8):
            B = c * 8 + blk
            lo, hi = int(bounds[B]), int(bounds[B + 1])
            off = blk * bcap
            esel[c, off:off + hi - lo] = np.arange(lo, hi)
            valid[c, off:off + hi - lo] = True
    esel_f = esel.reshape(-1)
    valid_f = valid.reshape(-1)

    nodecat = np.concatenate([node_memory, node_features], axis=1)  # [N,256]

    src_rows = nodecat[source_ids[esel_f]]
    src_rows[~valid_f] = 0.0
    # srcX[c, p, k, b, j] = src_rows[c, b*bcap+j, k*128+p]
    srcX = np.ascontiguousarray(
        src_rows.reshape(N_CORES, 8, bcap, 2, P).transpose(0, 4, 3, 1, 2)
    ).astype(FP8)

    ef_rows = edge_features[edge_ids[esel_f]]
    t_rows = time_encoding[np.minimum(esel_f, n_edge - 1)]
    eft = np.zeros((len(esel_f), P), np.float32)
    eft[:, :64] = ef_rows
    eft[:, 64:96] = t_rows
    eft[valid_f, 96] = 1.0
    eft[~valid_f, :96] = 0.0
    eftX = np.ascontiguousarray(
        eft.reshape(N_CORES, 8, bcap, P).transpose(0, 3, 1, 2)).astype(FP8)

    ld_e = (dest_seg[esel_f] % P).astype(np.int64)
    ld_e[~valid_f] = 255
    S_flat = (ld_e[:, None] == np.arange(P)[None, :]).astype(FP8)
    S_pack = np.ascontiguousarray(
        S_flat.reshape(N_CORES, 8, 4 * st_b, P, P).transpose(0, 3, 1, 2, 4))

    invc = np.ascontiguousarray(np.broadcast_to(
        inv_cnt.reshape(N_CORES, 1, 1024), (N_CORES, P, 1024))).astype(BF16)

    drows = nodecat[node_ids]                                  # [8192, 256]
    dstX = np.ascontiguousarray(
        drows.reshape(N_CORES, 1024, 2, P).transpose(0, 3, 2, 1)).astype(BF16)

    wrd_h = np.ascontiguousarray(
        W_read.reshape(2, P, P).transpose(1, 0, 2)).astype(FP8)
    wmg_k1 = np.zeros((P, P), np.float32)
    wmg_k1[:64] = W_msg[128:192]
    wmg_k1[64:96] = W_msg[192:224]
    wmg_k1[96] = b_msg
    wmg_h = np.ascontiguousarray(
        np.stack([W_msg[:P], wmg_k1], axis=0).transpose(1, 0, 2)).astype(FP8)
    wrb_h = np.ascontiguousarray(
        W_read.reshape(2, P, P).transpose(1, 0, 2)).astype(BF16)
    wa_h = np.ascontiguousarray(
        W_agg.reshape(2, P, P).transpose(1, 0, 2)).astype(BF16)
    wu_h = np.ascontiguousarray(
        W_upd.reshape(2, P, P).transpose(1, 0, 2)).astype(BF16)
    ww_h = np.ascontiguousarray(W_write).astype(BF16)
    br_h = np.ascontiguousarray(b_read[:, None]).astype(np.float32)
    ba_h = np.ascontiguousarray(b_agg[:, None]).astype(np.float32)
    bu_h = np.ascontiguousarray(b_upd[:, None]).astype(np.float32)
    bw_h = np.ascontiguousarray(b_write[:, None]).astype(np.float32)

    in_maps = []
    for c in range(N_CORES):
        in_maps.append({
            "srcX": srcX[c], "eftX": eftX[c], "S_d": S_pack[c],
            "invc": invc[c], "dstX": dstX[c],
            "wrd": wrd_h, "wmg": wmg_h, "wrb": wrb_h, "wa": wa_h, "wu": wu_h,
            "ww": ww_h, "br": br_h, "ba": ba_h, "bu": bu_h, "bw": bw_h,
        })
    return in_maps, st_b, node_memory, node_ids


def run(inputs, trace=False, **kw):
    in_maps, st_b, node_memory, node_ids = _prep_inputs(inputs)
    if st_b not in _PROG_CACHE:
        _PROG_CACHE[st_b] = _build_program(st_b)
    nc = _PROG_CACHE[st_b]
    res = run_bass_kernel_spmd(nc, in_maps, core_ids=list(range(N_CORES)),
                               trace=trace, **kw)
    wt = np.concatenate(
        [np.asarray(res.results[c]["writeT"], np.float32).T
         for c in range(N_CORES)], axis=0)             # [8192, 128]
    out = node_memory.copy()
    out[node_ids] = wt
    return out, res


def kernel(**inputs) -> np.ndarray:
    out, _ = run(inputs, trace=False)
    return out


# revision 11
# speedup vs baseline: 1.8717x; 1.2371x over previous
"""Trainium2 Bass kernel for LocalDualDirectedMessagePassingLayer.

Strategy (8 cores, dest-sharded, fp8 DoubleRow):
  - Each core owns 1024 destination segments (8 blocks of 128 dests).
  - dest_seg is sorted, so each dest block's edges are contiguous; host pads
    each block's edge list to BCAP = ST_B*512 and packs fp8e4 operands:
      srcX [128,2,8,bcap] = concat(node_memory,node_features)[source_ids].T
                            as two 128-row K-tiles (DoubleRow layout)
      eftX [128,8,bcap]   = [edge_feat(64); time(32); ones; zeros(31)].T
  - Read MLP: ONE DoubleRow fp8 matmul per 512-edge supertile (K=256),
    relu on ACT writes fp8 srcT into the shared activation slab.
  - Msg MLP: per 128-edge tile ONE DoubleRow matmul with lhsT =
    [srcT;eft] slab slice, rhs = W_msg K-tiles; relu f32->bf16 split
    across DVE/Pool.
  - Aggregation: one-hot S built ON DEVICE per tile via DVE 4x-mode
    tensor_scalar(iota == ldest) * scale (scale = 1/cnt, 0 for padding);
    DoubleRow... S/msgs are bf16 so agg matmuls are plain bf16 pairs
    accumulating msg_mean^T [msg,dest] in PSUM across the block.
  - Per block: dst-side MLP chain (read/agg/upd/write) in bf16 ->
    tanh -> writeT [128,1024] f32, interleaved with next block.
  - Host: transpose writeT, scatter rows into a copy of node_memory.
"""

import sys

sys.path.insert(0, "/opt/trn_rl_repo")

import math

import ml_dtypes
import numpy as np

import concourse.bass as bass
import concourse.mybir as mybir
import concourse.tile as tile
from concourse import bacc
from concourse.bass_utils import run_bass_kernel_spmd

BF16 = ml_dtypes.bfloat16
FP8 = ml_dtypes.float8_e4m3
N_CORES = 8
SUP = 512
P = 128
N_DEST = 8192
D_MEM = 128

_PROG_CACHE: dict[int, object] = {}


def _build_program(st_b: int):
    """SPMD Bass program; BCAP = st_b*512 padded edges per dest block."""
    bcap = st_b * SUP
    e_cap = 8 * bcap
    nt = e_cap // P

    nc = bacc.Bacc("TRN2", target_bir_lowering=False, debug=False,
                   num_devices=N_CORES)
    f32 = mybir.dt.float32
    bf16 = mybir.dt.bfloat16
    fp8 = mybir.dt.float8e4
    AF = mybir.ActivationFunctionType
    OP = mybir.AluOpType
    DR = mybir.MatmulPerfMode.DoubleRow

    srcX = nc.dram_tensor("srcX", [P, 2, 8, bcap], fp8, kind="ExternalInput")
    eftX = nc.dram_tensor("eftX", [P, 8, bcap], fp8, kind="ExternalInput")
    S_d = nc.dram_tensor("S_d", [P, 8, 4 * st_b, P], fp8, kind="ExternalInput")
    invc = nc.dram_tensor("invc", [P, 1024], bf16, kind="ExternalInput")
    dstX = nc.dram_tensor("dstX", [P, 2, 1024], bf16, kind="ExternalInput")
    wrd = nc.dram_tensor("wrd", [P, 2, P], fp8, kind="ExternalInput")
    wmg = nc.dram_tensor("wmg", [P, 2, P], fp8, kind="ExternalInput")
    wrb = nc.dram_tensor("wrb", [P, 2, P], bf16, kind="ExternalInput")
    wa = nc.dram_tensor("wa", [P, 2, P], bf16, kind="ExternalInput")
    wu = nc.dram_tensor("wu", [P, 2, P], bf16, kind="ExternalInput")
    ww = nc.dram_tensor("ww", [P, P], bf16, kind="ExternalInput")
    br = nc.dram_tensor("br", [P, 1], f32, kind="ExternalInput")
    ba = nc.dram_tensor("ba", [P, 1], f32, kind="ExternalInput")
    bu = nc.dram_tensor("bu", [P, 1], f32, kind="ExternalInput")
    bw = nc.dram_tensor("bw", [P, 1], f32, kind="ExternalInput")
    out_d = nc.dram_tensor("writeT", [P, 1024], f32, kind="ExternalOutput")

    with tile.TileContext(nc) as tc:
        with (
            tc.tile_pool(name="const", bufs=1) as cp,
            tc.tile_pool(name="src", bufs=3) as iop,
            tc.tile_pool(name="act", bufs=3) as actp,
            tc.tile_pool(name="sp", bufs=3) as sp,
            tc.tile_pool(name="mp", bufs=4) as mp,
            tc.tile_pool(name="mid", bufs=8) as midp,
            tc.tile_pool(name="work", bufs=5, space="PSUM") as workps,
            tc.tile_pool(name="agg", bufs=2, space="PSUM") as aggps,
            tc.tile_pool(name="dst", bufs=1, space="PSUM") as dstps,
        ):
            def cload(ap, shape, dtype, tag, eng=nc.sync):
                t = cp.tile(shape, dtype, tag=tag)
                eng.dma_start(out=t[:], in_=ap)
                return t

            invc_t = cload(invc[:, :], [P, 1024], bf16, "invc", nc.scalar)
            dstX_t = cload(dstX[:, :, :], [P, 2, 1024], bf16, "dstX", nc.scalar)
            wrd_t = cload(wrd[:, :, :], [P, 2, P], fp8, "wrd")
            wmg_t = cload(wmg[:, :, :], [P, 2, P], fp8, "wmg")
            wrb_t = cload(wrb[:, :, :], [P, 2, P], bf16, "wrb")
            wa_t = cload(wa[:, :, :], [P, 2, P], bf16, "wa")
            wu_t = cload(wu[:, :, :], [P, 2, P], bf16, "wu")
            ww_t = cload(ww[:, :], [P, P], bf16, "ww")
            br_t = cload(br[:, :], [P, 1], f32, "br", nc.scalar)
            ba_t = cload(ba[:, :], [P, 1], f32, "ba", nc.scalar)
            bu_t = cload(bu[:, :], [P, 1], f32, "bu", nc.scalar)
            bw_t = cload(bw[:, :], [P, 1], f32, "bw", nc.scalar)

            def dst_stage(b, agg_ps, stage, hold):
                dc = slice(b * P, (b + 1) * P)
                if stage == 0:
                    drp = dstps.tile([P, P], f32, tag="dst")
                    nc.tensor.matmul(drp[:], lhsT=wrb_t[:, 0, :],
                                     rhs=dstX_t[:, 0, dc],
                                     start=True, stop=False)
                    nc.tensor.matmul(drp[:], lhsT=wrb_t[:, 1, :],
                                     rhs=dstX_t[:, 1, dc],
                                     start=False, stop=True)
                    dstr = midp.tile([P, P], bf16, tag="dstr")
                    nc.scalar.activation(dstr[:], drp[:], AF.Relu, bias=br_t[:, :1])
                    hold.update(dstr=dstr)
                elif stage == 1:
                    agp = dstps.tile([P, P], f32, tag="dst")
                    nc.tensor.matmul(agp[:], lhsT=wa_t[:, 0, :], rhs=hold["dstr"][:],
                                     start=True, stop=False)
                    nc.tensor.matmul(agp[:], lhsT=wa_t[:, 1, :], rhs=hold["mmean"][:],
                                     start=False, stop=True)
                    aggT = midp.tile([P, P], bf16, tag="aggT")
                    nc.scalar.activation(aggT[:], agp[:], AF.Relu, bias=ba_t[:, :1])
                    hold.update(aggT=aggT)
                elif stage == 2:
                    upp = dstps.tile([P, P], f32, tag="dst")
                    nc.tensor.matmul(upp[:], lhsT=wu_t[:, 0, :], rhs=hold["aggT"][:],
                                     start=True, stop=False)
                    nc.tensor.matmul(upp[:], lhsT=wu_t[:, 1, :], rhs=hold["dstr"][:],
                                     start=False, stop=True)
                    updT = midp.tile([P, P], bf16, tag="updT")
                    nc.scalar.activation(updT[:], upp[:], AF.Relu, bias=bu_t[:, :1])
                    hold.update(updT=updT)
                else:
                    wrp = dstps.tile([P, P], f32, tag="dst")
                    nc.tensor.matmul(wrp[:], lhsT=ww_t[:], rhs=hold["updT"][:],
                                     start=True, stop=True)
                    wout = midp.tile([P, P], f32, tag="wout")
                    nc.scalar.activation(wout[:], wrp[:], AF.Tanh, bias=bw_t[:, :1])
                    nc.sync.dma_start(out=out_d[:, dc], in_=wout[:])

            nsup = 8 * st_b
            blocks = {}

            def load_block(b):
                srcB = iop.tile([P, 2, bcap], fp8, tag="src")
                nc.sync.dma_start(out=srcB[:], in_=srcX[:, :, b, :])
                actB = actp.tile([P, 2, bcap], fp8, tag="act")
                nc.sync.dma_start(out=actB[:, 1, :], in_=eftX[:, b, :])
                S_blk = sp.tile([P, 4 * st_b, P], fp8, tag="S")
                nc.sync.dma_start(out=S_blk[:], in_=S_d[:, b, :, :])
                blocks[b] = (srcB, actB, S_blk)

            def read_stage(g):
                """read MLP matmul + relu->fp8 for global supertile g."""
                b, st = divmod(g, st_b)
                cs = slice(st * SUP, (st + 1) * SUP)
                srcB, actB, _ = blocks[b]
                rd = workps.tile([P, SUP], f32, tag="work")
                nc.tensor.matmul(rd[:], lhsT=wrd_t[:], rhs=srcB[:, :, cs],
                                 start=True, stop=True, perf_mode=DR)
                nc.scalar.activation(actB[:, 0, cs], rd[:], AF.Relu,
                                     bias=br_t[:, :1])

            load_block(0)
            load_block(1)
            read_stage(0)
            read_stage(1)
            pending = None
            hold = {}
            agg_ps = None
            for g in range(nsup):
                b, st = divmod(g, st_b)
                if st == 0:
                    agg_ps = aggps.tile([P, P], f32, tag="agg")
                    if b + 2 < 8:
                        load_block(b + 2)
                # software pipeline: next supertile's read+relu first, so
                # srT is ready a full iteration before the msg matmuls
                if g + 2 < nsup:
                    read_stage(g + 2)

                _, actB, S_blk = blocks[b]
                mg = workps.tile([P, 4, P], f32, tag="work")
                for q in range(4):
                    qs = slice((st * 4 + q) * P, (st * 4 + q + 1) * P)
                    nc.tensor.matmul(mg[:, q, :], lhsT=actB[:, :, qs],
                                     rhs=wmg_t[:],
                                     start=True, stop=True, perf_mode=DR)
                msgs = mp.tile([P, 4, P], fp8, tag="msgs")
                for h in range(2):
                    hs = slice(2 * h, 2 * h + 2)
                    nc.vector.tensor_scalar(
                        out=msgs[:, hs, :], in0=mg[:, hs, :],
                        scalar1=0.0, scalar2=None, op0=OP.max)
                    nc.tensor.matmul(agg_ps[:], lhsT=msgs[:, hs, :],
                                     rhs=S_blk[:, st * 4 + 2 * h:st * 4 + 2 * h + 2, :],
                                     start=(st == 0 and h == 0),
                                     stop=(st == st_b - 1 and h == 1),
                                     perf_mode=DR)

                if pending is not None and st < 4:
                    dst_stage(pending[0], pending[1], st, hold)
                    if st == 3:
                        pending = None
                if st == st_b - 1:
                    mmean = midp.tile([P, P], bf16, tag="mmean")
                    nc.vector.tensor_tensor(mmean[:], agg_ps[:],
                                            invc_t[:, b * P:(b + 1) * P],
                                            OP.mult)
                    pending = (b, agg_ps)
                    hold = {"mmean": mmean}
            for stage in range(4):
                dst_stage(pending[0], pending[1], stage, hold)

    nc.finalize()
    return nc


def _prep_inputs(inputs):
    """Host-side shard/pack. Returns (in_maps, st_b, node_memory, node_ids)."""
    node_memory = np.ascontiguousarray(np.asarray(inputs["node_memory"], np.float32))
    node_features = np.asarray(inputs["node_features"], np.float32)
    edge_features = np.asarray(inputs["edge_features"], np.float32)
    time_encoding = np.asarray(inputs["time_encoding"], np.float32)
    node_ids = np.asarray(inputs["node_ids"]).astype(np.int64)
    source_ids = np.asarray(inputs["source_ids"]).astype(np.int64)
    edge_ids = np.asarray(inputs["edge_ids"]).astype(np.int64)
    dest_seg = np.asarray(inputs["dest_seg"]).astype(np.int64)
    W_read = np.asarray(inputs["W_read"], np.float32)
    b_read = np.asarray(inputs["b_read"], np.float32)
    W_msg = np.asarray(inputs["W_msg"], np.float32)
    b_msg = np.asarray(inputs["b_msg"], np.float32)
    W_agg = np.asarray(inputs["W_agg"], np.float32)
    b_agg = np.asarray(inputs["b_agg"], np.float32)
    W_upd = np.asarray(inputs["W_upd"], np.float32)
    b_upd = np.asarray(inputs["b_upd"], np.float32)
    W_write = np.asarray(inputs["W_write"], np.float32)
    b_write = np.asarray(inputs["b_write"], np.float32)

    n_edge = dest_seg.shape[0]

    cnt = np.bincount(dest_seg, minlength=N_DEST)
    inv_cnt = np.zeros(N_DEST, np.float32)
    nz = cnt > 0
    inv_cnt[nz] = 1.0 / cnt[nz]

    # 64 global dest blocks of 128; block B's edges are dest_seg in [B*128,(B+1)*128)
    bounds = np.searchsorted(dest_seg, np.arange(0, N_DEST + 1, P))
    per_block = np.diff(bounds)
    st_b = max(4, math.ceil(per_block.max() / SUP))
    bcap = st_b * SUP
    e_cap = 8 * bcap
    nt = e_cap // P

    # per-core edge selection (padded); esel indexes into the edge arrays
    esel = np.zeros((N_CORES, e_cap), np.int64)
    valid = np.zeros((N_CORES, e_cap), bool)
    for c in range(N_CORES):
        for blk in range(8):
            B = c * 8 + blk
            lo, hi = int(bounds[B]), int(bounds[B + 1])
            off = blk * bcap
            esel[c, off:off + hi - lo] = np.arange(lo, hi)
            valid[c, off:off + hi - lo] = True
    esel_f = esel.reshape(-1)
    valid_f = valid.reshape(-1)

    nodecat = np.concatenate([node_memory, node_features], axis=1)  # [N,256]

    src_rows = nodecat[source_ids[esel_f]]
    src_rows[~valid_f] = 0.0
    # srcX[c, p, k, b, j] = src_rows[c, b*bcap+j, k*128+p]
    srcX = np.ascontiguousarray(
        src_rows.reshape(N_CORES, 8, bcap, 2, P).transpose(0, 4, 3, 1, 2)
    ).astype(FP8)

    ef_rows = edge_features[edge_ids[esel_f]]
    t_rows = time_encoding[np.minimum(esel_f, n_edge - 1)]
    eft = np.zeros((len(esel_f), P), np.float32)
    eft[:, :64] = ef_rows
    eft[:, 64:96] = t_rows
    eft[valid_f, 96] = 1.0
    eft[~valid_f, :96] = 0.0
    eftX = np.ascontiguousarray(
        eft.reshape(N_CORES, 8, bcap, P).transpose(0, 3, 1, 2)).astype(FP8)

    ld_e = (dest_seg[esel_f] % P).astype(np.int64)
    ld_e[~valid_f] = 255
    S_flat = (ld_e[:, None] == np.arange(P)[None, :]).astype(FP8)
    S_pack = np.ascontiguousarray(
        S_flat.reshape(N_CORES, 8, 4 * st_b, P, P).transpose(0, 3, 1, 2, 4))

    invc = np.ascontiguousarray(np.broadcast_to(
        inv_cnt.reshape(N_CORES, 1, 1024), (N_CORES, P, 1024))).astype(BF16)

    drows = nodecat[node_ids]                                  # [8192, 256]
    dstX = np.ascontiguousarray(
        drows.reshape(N_CORES, 1024, 2, P).transpose(0, 3, 2, 1)).astype(BF16)

    wrd_h = np.ascontiguousarray(
        W_read.reshape(2, P, P).transpose(1, 0, 2)).astype(FP8)
    wmg_k1 = np.zeros((P, P), np.float32)
    wmg_k1[:64] = W_msg[128:192]
    wmg_k1[64:96] = W_msg[192:224]
    wmg_k1[96] = b_msg
    wmg_h = np.ascontiguousarray(
        np.stack([W_msg[:P], wmg_k1], axis=0).transpose(1, 0, 2)).astype(FP8)
    wrb_h = np.ascontiguousarray(
        W_read.reshape(2, P, P).transpose(1, 0, 2)).astype(BF16)
    wa_h = np.ascontiguousarray(
        W_agg.reshape(2, P, P).transpose(1, 0, 2)).astype(BF16)
    wu_h = np.ascontiguousarray(
        W_upd.reshape(2, P, P).transpose(1, 0, 2)).astype(BF16)
    ww_h = np.ascontiguousarray(W_write).astype(BF16)
    br_h = np.ascontiguousarray(b_read[:, None]).astype(np.float32)
    ba_h = np.ascontiguousarray(b_agg[:, None]).astype(np.float32)
    bu_h = np.ascontiguousarray(b_upd[:, None]).astype(np.float32)
    bw_h = np.ascontiguousarray(b_write[:, None]).astype(np.float32)

    in_maps = []
    for c in range(N_CORES):
        in_maps.append({
            "srcX": srcX[c], "eftX": eftX[c], "S_d": S_pack[c],
            "invc": invc[c], "dstX": dstX[c],
            "wrd": wrd_h, "wmg": wmg_h, "wrb": wrb_h, "wa": wa_h, "wu": wu_h,
            "ww": ww_h, "br": br_h, "ba": ba_h, "bu": bu_h, "bw": bw_h,
        })
    return in_maps, st_b, node_memory, node_ids


def run(inputs, trace=False, **kw):
    in_maps, st_b, node_memory, node_ids = _prep_inputs(inputs)
    if st_b not in _PROG_CACHE:
        _PROG_CACHE[st_b] = _build_program(st_b)
    nc = _PROG_CACHE[st_b]
    res = run_bass_kernel_spmd(nc, in_maps, core_ids=list(range(N_CORES)),
                               trace=trace, **kw)
    wt = np.concatenate(
        [np.asarray(res.results[c]["writeT"], np.float32).T
         for c in range(N_CORES)], axis=0)             # [8192, 128]
    out = node_memory.copy()
    out[node_ids] = wt
    return out, res


def kernel(**inputs) -> np.ndarray:
    out, _ = run(inputs, trace=False)
    return out
